# revision 26
# baseline (speedup 1.0000x reference)
"""Trainium2 Bass kernel: sigmoid multi-head attention (16 heads, S=2048, D=1024,
P=64) + final linear, head-sharded across 8 NeuronCores (2 heads/core).

Reference semantics: concat = attn.reshape(S, -1) is a RAW reshape of the
contiguous [H, S, P] attn array, so output row i draws only from head
h = i // 128:  out[h*128 + r, f] = sum_{u,p} attn[h, 16r+u, p] * W_fin[u*64+p, f].
Core c (heads 2c, 2c+1) therefore owns output rows [256c, 256c+256) exactly;
the host gather is a concatenation (no cross-core reduction).

Per-core plan (heads h0,h1 stacked on partitions as p2 = h*64 + p):
  1. QT2/KT2/VT2 [128, S] = Wl.T @ xT  (fp32r matmuls, K=128 d-chunks, N=512)
  2. V natural [t, p2] via PE transpose of VT2 128x128 tiles
  3. for each s-block pair q (2x512 cols of s):
       per s-block: 16 t-tiles: scoreT psum [128t, 1024] = two row-tiled
       concurrent MMs (K=64, h0|h1); sigmoid(x/64) on ScalarE -> SBUF;
       attnT psum [128, 512] += two col-tiled concurrent MMs.
       after both s-blocks: final linear with strided-column lhsT from the
       attnT staging tile and a partition-duplicated W_fin (wf2), giving
       64 output rows per head; copy + DMA out.
"""

import os

os.environ.setdefault("BASS_NEVER_TRACE", "1")

import numpy as np
from contextlib import ExitStack

import jax
import concourse.bacc as bacc
import concourse.bass as bass
import concourse.mybir as mybir
import concourse.tile as tile
from concourse.bass2jax import (
    _bass_exec_p,
    install_neuronx_cc_hook,
    partition_id_tensor,
)
from jax.experimental.shard_map import shard_map
from jax.sharding import Mesh, NamedSharding, PartitionSpec

S, D, H, P, F = 2048, 1024, 16, 64, 1024
NCORES = 8
HL = H // NCORES          # heads per core = 2
P2 = HL * P               # stacked head dim = 128
DCH = D // 128            # 8 contraction chunks
NSB = S // 512            # 4 s-blocks
NT = S // 128             # 16 t-tiles
NU = 16                   # final-linear contraction sub-chunks (u = s % 16)

FP32 = mybir.dt.float32
FP32R = mybir.dt.float32r
SIGMOID = mybir.ActivationFunctionType.Sigmoid


def build_kernel(ctx: ExitStack, tc: tile.TileContext, xt_d, wq_d, wk_d, wv_d,
                 wf2_d, id_d, out_d):
    nc = tc.nc

    const_pool = ctx.enter_context(tc.tile_pool(name="const", bufs=1))
    w_pool = ctx.enter_context(tc.tile_pool(name="wts", bufs=1))
    qk_pool = ctx.enter_context(tc.tile_pool(name="qk", bufs=1))

    ident = const_pool.tile([128, 128], FP32, tag="ident")
    nc.gpsimd.dma_start(ident, id_d)

    qt2 = qk_pool.tile([128, S], FP32R, tag="qt2")   # [p2, s]
    kt2 = qk_pool.tile([128, S], FP32R, tag="kt2")   # [p2, t]
    v2n = qk_pool.tile([128, S], FP32R, tag="v2n")   # [t_in, j*128 + p2]

    # PSUM plan (8 banks): stage-P projections use 2 (pj + pt), scores 4
    # (ps_s double-buffered [128,1024]), attn 2 (per-head banks). The final
    # stage's 2 banks are allocated from the released stage-P zone — its
    # overlap-dep on stage-P retirement is harmless since the first final
    # matmul runs long after the projections finish.
    pp_pool = tc.alloc_tile_pool(name="pp", bufs=1, space="PSUM", side="right")
    ps_s_pool = ctx.enter_context(tc.tile_pool(name="ps_s", bufs=2, space="PSUM"))
    ps_a_pool = ctx.enter_context(tc.tile_pool(name="ps_a", bufs=1, space="PSUM"))
    # SBUF pools used during the score stream are allocated BEFORE the
    # transient xt/vt pools so they never land in a released stage-P zone
    # (which would add blocking deps on stage-P retirement).
    sc_pool = ctx.enter_context(tc.tile_pool(name="sc", bufs=8))
    att_pool = ctx.enter_context(tc.tile_pool(name="att", bufs=2))
    ot_pool = ctx.enter_context(tc.tile_pool(name="ot", bufs=4))

    # ---------------- stage P: projections, s-column-streamed ----------------
    # xt arrives as 32 [128,512] column-block tiles, s-block-major, so the
    # s-block 0 projections (and with them the score/sigmoid stream) start
    # ~6us in instead of waiting for the full 8 MB of x.
    with tc.tile_pool(name="xt", bufs=2) as xt_pool, \
         tc.tile_pool(name="vt", bufs=1) as vt_pool:

        # small weight DMAs go to the SWDGE queues so the xt column stream
        # owns all 8 HWDGE queues from t=0
        wq = w_pool.tile([128, D], FP32R, tag="wq")
        nc.gpsimd.dma_start(wq, wq_d.bitcast(FP32R))
        wk = w_pool.tile([128, D], FP32R, tag="wk")
        nc.gpsimd.dma_start(wk, wk_d.bitcast(FP32R))
        wv = w_pool.tile([128, D], FP32R, tag="wv")
        nc.gpsimd.dma_start(wv, wv_d.bitcast(FP32R))

        xts = [[None] * DCH for _ in range(NSB)]
        for sb in range(NSB):
            for d in range(DCH):
                xt_t = xt_pool.tile([128, 512], FP32R, tag=f"x{d}",
                                    name=f"xt{sb}_{d}")
                nc.sync.dma_start(
                    xt_t,
                    xt_d[d * 128:(d + 1) * 128,
                         sb * 512:(sb + 1) * 512].bitcast(FP32R))
                xts[sb][d] = xt_t

        # wf2 [128, NU*F]: W_fin chunk u duplicated on both partition halves.
        # Emitted after xt so the x stream wins the DMA queues.
        wf2 = w_pool.tile([128, NU * F], FP32R, tag="wf2")
        for i in range(8):
            w = NU * F // 8
            nc.sync.dma_start(wf2[:, i * w:(i + 1) * w],
                              wf2_d[:, i * w:(i + 1) * w].bitcast(FP32R))

        vt2 = vt_pool.tile([128, S], FP32, tag="vt2")

        for sb in range(NSB):
            cols = slice(sb * 512, (sb + 1) * 512)
            for w, dst in ((wk, kt2), (wq, qt2), (wv, None)):
                ps_p = pp_pool.tile([128, 512], FP32, tag="pj",
                                    name=f"pp{sb}_{0 if dst is kt2 else (1 if dst is qt2 else 2)}")
                for d in range(DCH):
                    nc.tensor.matmul(ps_p, w[:, d * 128:(d + 1) * 128],
                                     xts[sb][d],
                                     start=(d == 0), stop=(d == DCH - 1))
                if dst is None:
                    nc.vector.tensor_copy(vt2[:, cols], ps_p)
                else:
                    nc.vector.tensor_copy(dst[:, cols], ps_p)
            # transpose the 4 fresh VT2 tiles -> V natural
            for j in range(4 * sb, 4 * sb + 4):
                pt = pp_pool.tile([128, 128], FP32, tag="pt", name=f"pt{j}")
                nc.tensor.transpose(pt, vt2[:, j * 128:(j + 1) * 128], ident)
                nc.vector.tensor_copy(v2n[:, j * 128:(j + 1) * 128], pt)

    pp_pool.release()

    # ---------------- stage S + F: scores / attn / final ----------------
    # All matmul outputs start at PSUM partition 0 (col groups at nonzero
    # base fail walrus codegen). Attn runs one M=128 matmul per head with
    # the full 128-col V tile as stationary: head h's valid rows land at
    # partitions h*64..h*64+64 (the other half is discarded), keeping the
    # attnT staging layout partition-aligned for the final stage.
    with tc.tile_pool(name="ps_f", bufs=1, space="PSUM", side="right") as ps_f_pool:

        for q in range(NSB // 2):
            at = att_pool.tile([128, 1024], FP32R, tag="at", name=f"at{q}")
            for half in range(2):
                sb = 2 * q + half
                s0 = sb * 512
                ps_a0 = ps_a_pool.tile([128, 512], FP32, tag="ah0",
                                       name=f"ps_a0_{sb}")
                ps_a1 = ps_a_pool.tile([128, 512], FP32, tag="ah1",
                                       name=f"ps_a1_{sb}")
                for j in range(NT):
                    t0 = j * 128
                    ps_s = ps_s_pool.tile([128, 1024], FP32, tag="ps_s",
                                          name=f"ps_s{sb}_{j}")
                    # scoreT h0 / h1 — concurrent on PE via row groups 0 / 64
                    nc.tensor.matmul(ps_s[:, 0:512],
                                     kt2[0:64, t0:t0 + 128],
                                     qt2[0:64, s0:s0 + 512])
                    nc.tensor.matmul(ps_s[:, 512:1024],
                                     kt2[64:128, t0:t0 + 128],
                                     qt2[64:128, s0:s0 + 512])
                    sc = sc_pool.tile([128, 1024], FP32R, tag="sc",
                                      name=f"sc{sb}_{j}")
                    nc.scalar.activation(sc, ps_s, SIGMOID, scale=1.0 / P)
                    # attnT accumulation, shared stationary V tile per j
                    nc.tensor.matmul(ps_a0,
                                     v2n[:, t0:t0 + 128],
                                     sc[:, 0:512],
                                     start=(j == 0), stop=(j == NT - 1))
                    nc.tensor.matmul(ps_a1,
                                     v2n[:, t0:t0 + 128],
                                     sc[:, 512:1024],
                                     start=(j == 0), stop=(j == NT - 1))
                nc.vector.tensor_copy(at[0:64, half * 512:(half + 1) * 512],
                                      ps_a0[0:64, :])
                nc.vector.tensor_copy(at[64:128, half * 512:(half + 1) * 512],
                                      ps_a1[64:128, :])

            # final linear for this s-block pair: 64 output rows per head;
            # h0 (rows 0:64) and h1 (rows 64:128) run on disjoint row groups
            for fc in range(2):
                psf0 = ps_f_pool.tile([64, 512], FP32, tag="pf0",
                                      name=f"psf0_{q}_{fc}")
                psf1 = ps_f_pool.tile([64, 512], FP32, tag="pf1",
                                      name=f"psf1_{q}_{fc}")
                for u in range(NU):
                    fcol = u * F + fc * 512
                    nc.tensor.matmul(psf0, at[0:64, u::NU],
                                     wf2[0:64, fcol:fcol + 512],
                                     start=(u == 0), stop=(u == NU - 1))
                    nc.tensor.matmul(psf1, at[64:128, u::NU],
                                     wf2[64:128, fcol:fcol + 512],
                                     start=(u == 0), stop=(u == NU - 1))
                for h, psf in ((0, psf0), (1, psf1)):
                    otf = ot_pool.tile([64, 512], FP32, tag="ot",
                                       name=f"ot{q}_{fc}_{h}")
                    nc.vector.tensor_copy(otf, psf)
                    nc.sync.dma_start(
                        out_d[h * 128 + 64 * q: h * 128 + 64 * (q + 1),
                              fc * 512:(fc + 1) * 512],
                        otf)


def build_bass(replicas: int = 1) -> bass.Bass:
    nc = bacc.Bacc("TRN2", target_bir_lowering=False, debug=False,
                   num_devices=NCORES)
    xt_d = nc.dram_tensor("xt", [D, S], FP32, kind="ExternalInput").ap()
    wq_d = nc.dram_tensor("wq", [128, D], FP32, kind="ExternalInput").ap()
    wk_d = nc.dram_tensor("wk", [128, D], FP32, kind="ExternalInput").ap()
    wv_d = nc.dram_tensor("wv", [128, D], FP32, kind="ExternalInput").ap()
    wf2_d = nc.dram_tensor("wf2", [128, NU * F], FP32, kind="ExternalInput").ap()
    id_d = nc.dram_tensor("ident", [128, 128], FP32, kind="ExternalInput").ap()
    out_d = nc.dram_tensor("out", [HL * 128, F], FP32, kind="ExternalOutput").ap()
    with tile.TileContext(nc) as tc:
        for _ in range(replicas):
            with ExitStack() as ctx:
                build_kernel(ctx, tc, xt_d, wq_d, wk_d, wv_d, wf2_d, id_d,
                             out_d)
    nc.finalize()
    return nc


_NC_CACHE = None
_EXEC_CACHE = None
LAST_DEV_ARGS = None
LAST_OUT_NAMES = None


def _get_nc():
    global _NC_CACHE
    if _NC_CACHE is None:
        _NC_CACHE = build_bass()
    return _NC_CACHE


def _get_executor():
    """Compile the SPMD PJRT executable once (mirrors bass2jax.run_bass_via_pjrt,
    minus output-buffer donation — every output element is written by the kernel,
    so inputs can stay device-resident across repeated timed executions)."""
    global _EXEC_CACHE
    if _EXEC_CACHE is not None:
        return _EXEC_CACHE
    import concourse.mybir as mybir

    nc = _get_nc()
    install_neuronx_cc_hook()
    partition_name = (nc.partition_id_tensor.name
                      if nc.partition_id_tensor else None)
    in_names, out_names, out_avals = [], [], []
    out_shapes = []
    for alloc in nc.m.functions[0].allocations:
        if not isinstance(alloc, mybir.MemoryLocationSet):
            continue
        name = alloc.memorylocations[0].name
        if alloc.kind == "ExternalInput":
            if name != partition_name:
                in_names.append(name)
        elif alloc.kind == "ExternalOutput":
            shape = tuple(alloc.tensor_shape)
            dtype = mybir.dt.np(alloc.dtype)
            out_names.append(name)
            out_avals.append(jax.core.ShapedArray(shape, dtype))
            out_shapes.append((shape, dtype))
    n_params = len(in_names)
    all_names = list(in_names) + list(out_names)
    if partition_name is not None:
        all_names.append(partition_name)

    def _body(*args):
        operands = list(args)
        if partition_name is not None:
            operands.append(partition_id_tensor())
        outs = _bass_exec_p.bind(
            *operands,
            out_avals=tuple(out_avals),
            in_names=tuple(all_names),
            out_names=tuple(out_names),
            lowering_input_output_aliases=(),
            sim_require_finite=True,
            sim_require_nnan=True,
            nc=nc,
        )
        return tuple(outs)

    devices = jax.devices()[:NCORES]
    mesh = Mesh(np.asarray(devices), ("core",))
    n_args = n_params + len(out_names)
    sharded = jax.jit(shard_map(
        _body, mesh=mesh,
        in_specs=(PartitionSpec("core"),) * n_args,
        out_specs=(PartitionSpec("core"),) * len(out_names),
        check_rep=False))
    _EXEC_CACHE = (sharded, mesh, in_names, out_names, out_shapes)
    return _EXEC_CACHE


def _run_spmd(in_maps):
    """Execute on all cores; returns list of per-core {name: np.ndarray}."""
    global LAST_DEV_ARGS, LAST_OUT_NAMES
    sharded, mesh, in_names, out_names, out_shapes = _get_executor()
    sh = NamedSharding(mesh, PartitionSpec("core"))
    args = [np.concatenate([im[name] for im in in_maps], axis=0)
            for name in in_names]
    for shape, dtype in out_shapes:
        args.append(np.zeros((NCORES * shape[0],) + shape[1:], dtype))
    dev_args = [jax.device_put(a, sh) for a in args]
    LAST_DEV_ARGS = dev_args
    LAST_OUT_NAMES = out_names
    outs = sharded(*dev_args)
    jax.block_until_ready(outs)
    results = []
    for c in range(NCORES):
        res = {}
        for i, name in enumerate(out_names):
            g = np.asarray(outs[i])
            d0 = g.shape[0] // NCORES
            res[name] = g[c * d0:(c + 1) * d0]
        results.append(res)
    return results


def bench(iters=32):
    """Re-run the last-executed kernel `iters` times on device-resident inputs;
    returns per-iteration wall time in seconds (dispatch-pipelined)."""
    import time
    sharded = _get_executor()[0]
    assert LAST_DEV_ARGS is not None, "call kernel() first"
    outs = sharded(*LAST_DEV_ARGS)
    jax.block_until_ready(outs)
    t0 = time.perf_counter()
    pend = [sharded(*LAST_DEV_ARGS) for _ in range(iters)]
    jax.block_until_ready(pend)
    return (time.perf_counter() - t0) / iters


_FAST_CACHE = None


def _get_fast():
    """Fast-dispatch (effect-suppressed, C++ dispatch path) compile of the
    same SPMD executable, for benchmarking."""
    global _FAST_CACHE
    if _FAST_CACHE is not None:
        return _FAST_CACHE
    from concourse.bass2jax import fast_dispatch_compile
    import concourse.mybir as mybir

    nc = _get_nc()
    install_neuronx_cc_hook()
    partition_name = (nc.partition_id_tensor.name
                      if nc.partition_id_tensor else None)
    in_names, out_names, out_avals = [], [], []
    for alloc in nc.m.functions[0].allocations:
        if not isinstance(alloc, mybir.MemoryLocationSet):
            continue
        name = alloc.memorylocations[0].name
        if alloc.kind == "ExternalInput":
            if name != partition_name:
                in_names.append(name)
        elif alloc.kind == "ExternalOutput":
            out_names.append(name)
            out_avals.append(jax.core.ShapedArray(
                tuple(alloc.tensor_shape), mybir.dt.np(alloc.dtype)))
    n_params = len(in_names)
    all_names = list(in_names) + list(out_names)
    if partition_name is not None:
        all_names.append(partition_name)

    def _body(*args):
        operands = list(args)
        if partition_name is not None:
            operands.append(partition_id_tensor())
        return tuple(_bass_exec_p.bind(
            *operands,
            out_avals=tuple(out_avals),
            in_names=tuple(all_names),
            out_names=tuple(out_names),
            lowering_input_output_aliases=(),
            sim_require_finite=True,
            sim_require_nnan=True,
            nc=nc,
        ))

    devices = jax.devices()[:NCORES]
    mesh = Mesh(np.asarray(devices), ("core",))
    n_args = n_params + len(out_names)
    assert LAST_DEV_ARGS is not None

    def compile_fn():
        return jax.jit(shard_map(
            _body, mesh=mesh,
            in_specs=(PartitionSpec("core"),) * n_args,
            out_specs=(PartitionSpec("core"),) * len(out_names),
            check_rep=False)).lower(*LAST_DEV_ARGS).compile()

    _FAST_CACHE = fast_dispatch_compile(compile_fn)
    return _FAST_CACHE


def bench_fast(iters=64):
    """Per-iteration device time with C++ fast dispatch, async-queued.
    Includes ~0.3-0.4 ms of per-execute dispatch/NEFF-invocation overhead."""
    import time
    fn = _get_fast()
    assert LAST_DEV_ARGS is not None
    outs = fn(*LAST_DEV_ARGS)
    jax.block_until_ready(outs)
    best = None
    for _ in range(3):
        t0 = time.perf_counter()
        pend = [fn(*LAST_DEV_ARGS) for _ in range(iters)]
        jax.block_until_ready(pend)
        dt = (time.perf_counter() - t0) / iters
        best = dt if best is None else min(best, dt)
    return best


def _make_fast_for(nc):
    from concourse.bass2jax import fast_dispatch_compile
    import concourse.mybir as mybir

    install_neuronx_cc_hook()
    pn = nc.partition_id_tensor.name if nc.partition_id_tensor else None
    in_names, out_names, out_avals = [], [], []
    for alloc in nc.m.functions[0].allocations:
        if not isinstance(alloc, mybir.MemoryLocationSet):
            continue
        name = alloc.memorylocations[0].name
        if alloc.kind == "ExternalInput":
            if name != pn:
                in_names.append(name)
        elif alloc.kind == "ExternalOutput":
            out_names.append(name)
            out_avals.append(jax.core.ShapedArray(
                tuple(alloc.tensor_shape), mybir.dt.np(alloc.dtype)))
    all_names = list(in_names) + list(out_names)
    if pn:
        all_names.append(pn)

    def _body(*a):
        ops = list(a)
        if pn:
            ops.append(partition_id_tensor())
        return tuple(_bass_exec_p.bind(
            *ops, out_avals=tuple(out_avals), in_names=tuple(all_names),
            out_names=tuple(out_names), lowering_input_output_aliases=(),
            sim_require_finite=True, sim_require_nnan=True, nc=nc))

    mesh = Mesh(np.asarray(jax.devices()[:NCORES]), ("core",))
    na = len(in_names) + len(out_names)

    def cf():
        return jax.jit(shard_map(
            _body, mesh=mesh,
            in_specs=(PartitionSpec("core"),) * na,
            out_specs=(PartitionSpec("core"),) * len(out_names),
            check_rep=False)).lower(*LAST_DEV_ARGS).compile()

    return fast_dispatch_compile(cf)


def bench_body(iters=1024, reps=5):
    """True kernel-body execution time: difference between a single-body and a
    double-body (same I/O, body emitted twice) NEFF, measured over long
    async-queued runs so fixed dispatch overhead cancels."""
    import time
    import statistics
    assert LAST_DEV_ARGS is not None, "call kernel() first"
    fn1 = _get_fast()
    fn2 = _make_fast_for(build_bass(replicas=2))
    jax.block_until_ready(fn1(*LAST_DEV_ARGS))
    jax.block_until_ready(fn2(*LAST_DEV_ARGS))

    def run(fn, n):
        t0 = time.perf_counter()
        pend = [fn(*LAST_DEV_ARGS) for _ in range(n)]
        jax.block_until_ready(pend)
        return time.perf_counter() - t0

    run(fn1, 32)
    run(fn2, 32)
    t1s, t2s = [], []
    for _ in range(reps):
        t1s.append(run(fn1, iters))
        t2s.append(run(fn2, iters))
    return (statistics.median(t2s) - statistics.median(t1s)) / iters


def _layout_w(w, c):
    """[H, D, P] global weights -> per-core [128, D] stationary layout:
    out[di, dc*128 + (h*64+p)] = w[2c+h, dc*128+di, p]"""
    wl = np.transpose(w[HL * c:HL * (c + 1)], (1, 0, 2)).reshape(D, P2)
    wl = wl.reshape(DCH, 128, P2).transpose(1, 0, 2).reshape(128, DCH * P2)
    return np.ascontiguousarray(wl, dtype=np.float32)


def kernel(x, Qw, Kw, Vw, W_fin, b_fin):
    x = np.asarray(x, dtype=np.float32)
    Qw = np.asarray(Qw, dtype=np.float32)
    Kw = np.asarray(Kw, dtype=np.float32)
    Vw = np.asarray(Vw, dtype=np.float32)
    W_fin = np.asarray(W_fin, dtype=np.float32)
    b_fin = np.asarray(b_fin, dtype=np.float32)

    xt = np.ascontiguousarray(x.T)
    ident = np.eye(128, dtype=np.float32)
    # wf2: chunk u (64 rows of W_fin) duplicated on both partition halves
    wr = np.transpose(W_fin.reshape(NU, 64, F), (1, 0, 2))   # [64, NU, F]
    wf2 = np.empty((128, NU, F), dtype=np.float32)
    wf2[0:64] = wr
    wf2[64:128] = wr
    wf2 = np.ascontiguousarray(wf2.reshape(128, NU * F))

    in_maps = []
    for c in range(NCORES):
        in_maps.append({
            "xt": xt,
            "wq": _layout_w(Qw, c),
            "wk": _layout_w(Kw, c),
            "wv": _layout_w(Vw, c),
            "wf2": wf2,
            "ident": ident,
        })
    results = _run_spmd(in_maps)
    out = np.concatenate([results[c]["out"] for c in range(NCORES)], axis=0)
    return (out + b_fin).astype(np.float32)
